# revision 1
# baseline (speedup 1.0000x reference)
"""Trainium2 Bass kernel for nn_CRAU (per-channel sparse attention).

Computation (per batch b, channel c):
  qc  = Wq @ src (1x1 conv; bias folded into the S-reduction seed)
  S[c,t] = sum_d unfold(qc)[c,t,d] * feat[c,d] * (1/64)      t in 3x3 window
  A   = softmax_t(S)
  vc  = Wv @ feat + bv (1x1 conv)
  out = fold(A outer vc) * src

Sharding: 8 cores = 4 batches x 2 spatial halves (rows). The q.k reduction
is spatially partial per core; a pairwise AllReduce of S ([128,9] f32 per
channel-half, issued as soon as that half's partials finish so softmax/fold
of one half overlaps the reduction/compute of the other) links the two
halves of each batch.

The fold/unfold (3x3, stride 2, pad 1) is decomposed into 4 output parity
classes, each a small per-channel linear combination of shifted vc planes,
executed with fused DVE scalar_tensor_tensor ops, ScalarE per-partition-
scale multiplies, and GpSimd tensor-tensor multiplies. Convs run on the PE
in fp16 (full rate); the q.k reduction uses the custom-DVE
TENSOR_TENSOR_REDUCE op reading a column-parity-split qc so most of the 9
window offsets stream with unit stride.
"""

import numpy as np

N_CORES = 8
SRC_R, SRC_C = 65, 129          # per-core src slab (padded rows/cols)
FEAT_R, FEAT_C = 33, 66         # per-core feat slab (padded, even width)
SRCN = SRC_R * SRC_C            # 8385
FEATN = FEAT_R * FEAT_C         # 2178
QE_C, QO_C = 66, 64             # qc even-col / odd-col tile widths
OUTN = 64 * 128                 # per-core output elements per channel
SCALE = 1.0 / 64.0
QROWS = 13                      # q-conv rows per PSUM chunk (13*129=1677)

_prog_cache = {}
TRACE = False
TRACE_KW = {}
LAST_RESULT = [None]
STAGE = [99]


def _build(add_bv: bool, stage: int = 99):
    import concourse.mybir as mybir
    import concourse.tile as tile
    from concourse import bacc
    from concourse.dve_ops import TENSOR_TENSOR_REDUCE

    f32 = mybir.dt.float32
    f16 = mybir.dt.float16
    ADD = mybir.AluOpType.add
    MULT = mybir.AluOpType.mult
    MAX = mybir.AluOpType.max
    AX = mybir.AxisListType.X
    Exp = mybir.ActivationFunctionType.Exp

    nc = bacc.Bacc("TRN2", target_bir_lowering=False, debug=False,
                   num_devices=N_CORES)

    src_d = nc.dram_tensor("src", [256, SRCN], f32, kind="ExternalInput").ap()
    feat_d = nc.dram_tensor("feat", [256, FEATN], f32, kind="ExternalInput").ap()
    wpack_d = nc.dram_tensor("wpack", [256, 512], f32, kind="ExternalInput").ap()
    sinit_d = nc.dram_tensor("s_init", [256, 9], f32, kind="ExternalInput").ap()
    bv_d = nc.dram_tensor("bv", [256, 1], f32, kind="ExternalInput").ap()
    out_d = nc.dram_tensor("out", [256, OUTN], f32, kind="ExternalOutput").ap()

    with tile.TileContext(nc) as tc:
        with (
            tc.tile_pool(name="srcp", bufs=2) as srcp,
            tc.tile_pool(name="featp", bufs=2) as featp,
            tc.tile_pool(name="vcp", bufs=2) as vcp,
            tc.tile_pool(name="qcp", bufs=1) as qcp,
            tc.tile_pool(name="constp", bufs=2) as constp,
            tc.tile_pool(name="smp", bufs=1) as smp,
            tc.tile_pool(name="tup", bufs=4) as tup,
            tc.tile_pool(name="outp", bufs=2) as outp,
            tc.tile_pool(name="ps", bufs=2, space="PSUM") as ps,
            tc.tile_pool(name="dramp", bufs=2, space="DRAM") as dramp,
        ):
            # ---- loads (chunked so compute starts early) ----
            src_t = []
            feat_t = []
            w_t = []
            for h in range(2):
                wt = constp.tile([128, 512], f16, tag="w")
                nc.gpsimd.dma_start(wt[:], wpack_d[128 * h:128 * h + 128, :])
                w_t.append(wt)
            for h in range(2):
                st = srcp.tile([128, SRCN], f16, tag="src")
                for c0 in range(0, SRCN, 2145):
                    csz = min(2145, SRCN - c0)
                    nc.gpsimd.dma_start(
                        st[:, c0:c0 + csz],
                        src_d[128 * h:128 * h + 128, c0:c0 + csz])
                src_t.append(st)
                ft = featp.tile([128, FEATN], f16, tag="feat")
                for c0 in range(0, FEATN, 1089):
                    nc.gpsimd.dma_start(
                        ft[:, c0:c0 + 1089],
                        feat_d[128 * h:128 * h + 128, c0:c0 + 1089])
                feat_t.append(ft)

            # smalls layout (cols):
            # [0:9] S(h0) [9:18] S(h1) [18:27] Ssum(h0) [27:36] Ssum(h1)
            # [36:45] A(h0) [45:54] A(h1) [54:63] E scratch
            # [63:64] m [64:65] nm [65:66] sum [66:67] r
            # [68:77] sinit(h0) [77:86] sinit(h1)  [86:88] bv(h0,h1)
            sm = smp.tile([128, 96], f32, tag="smalls")
            for h in range(2):
                nc.sync.dma_start(sm[:, 68 + 9 * h:77 + 9 * h],
                                  sinit_d[128 * h:128 * h + 128, :])
                if add_bv:
                    nc.sync.dma_start(sm[:, 86 + h:87 + h],
                                      bv_d[128 * h:128 * h + 128, :])

            # ---- v-conv (frees PSUM early; vc needed only for fold) ----
            vc_t = []
            for h in range(2 if stage >= 1 else 0):
                vt = vcp.tile([128, FEATN], f16, tag="vc")
                for c0 in range(0, FEATN, 2048):
                    csz = min(2048, FEATN - c0)
                    pt = ps.tile([128, 2048], f32, tag="mm")
                    for s0 in range(0, csz, 512):
                        ssz = min(512, csz - s0)
                        for kt in range(2):
                            nc.tensor.matmul(
                                pt[:, s0:s0 + ssz],
                                lhsT=w_t[kt][:, 256 + 128 * h:256 + 128 * h + 128],
                                rhs=feat_t[kt][:, c0 + s0:c0 + s0 + ssz],
                                start=(kt == 0), stop=(kt == 1))
                    if add_bv:
                        nc.vector.tensor_scalar(
                            out=vt[:, c0:c0 + csz], in0=pt[:, 0:csz],
                            scalar1=sm[:, 86 + h:87 + h], scalar2=None,
                            op0=ADD)
                    else:
                        nc.scalar.copy(vt[:, c0:c0 + csz], pt[:, 0:csz])
                if add_bv:
                    v3 = vt.rearrange("p (r q) -> p r q", q=FEAT_C)
                    nc.gpsimd.memset(v3[:, FEAT_R - 1, :], 0.0)
                    nc.gpsimd.memset(v3[:, :, 64:66], 0.0)
                vc_t.append(vt)

            # ---- q-conv + S partials + per-half collective ----
            S_b = []
            S_r = []
            for h in range(2):
                sbt = dramp.tile([128, 9], f32, tag=f"sb{h}", name=f"sb{h}")
                srt = dramp.tile([128, 9], f32, tag=f"sr{h}", name=f"sr{h}")
                S_b.append(sbt)
                S_r.append(srt)
            for h in range(2 if stage >= 2 else 0):
                # qc column-parity-split tiles:
                #   qe[r, m] = qc[r, 2m]   (m in [0,65), row width QE_C=66)
                #   qo[r, m] = qc[r, 2m+1] (m in [0,64))
                qe = qcp.tile([128, SRC_R * QE_C], f16, tag="qe")
                qo = qcp.tile([128, SRC_R * QO_C], f16, tag="qo")
                qe3 = qe.rearrange("p (r q) -> p r q", q=QE_C)
                qo3 = qo.rearrange("p (r q) -> p r q", q=QO_C)
                # row-aligned PSUM chunks of QROWS src rows each
                for r0 in range(0, SRC_R, QROWS):
                    nrow = min(QROWS, SRC_R - r0)
                    csz = nrow * SRC_C
                    c0 = r0 * SRC_C
                    pt = ps.tile([128, 2048], f32, tag="mm")
                    for s0 in range(0, csz, 512):
                        ssz = min(512, csz - s0)
                        for kt in range(2):
                            nc.tensor.matmul(
                                pt[:, s0:s0 + ssz],
                                lhsT=w_t[kt][:, 128 * h:128 * h + 128],
                                rhs=src_t[kt][:, c0 + s0:c0 + s0 + ssz],
                                start=(kt == 0), stop=(kt == 1))
                    pt3 = pt[:, 0:csz].rearrange("p (r q) -> p r q", q=SRC_C)
                    nc.scalar.copy(qe3[:, r0:r0 + nrow, 0:65],
                                   pt3[:, :, 0:129:2])
                    nc.scalar.copy(qo3[:, r0:r0 + nrow, 0:64],
                                   pt3[:, :, 1:128:2])

                k3 = feat_t[h].rearrange("p (r q) -> p r q", q=FEAT_C)
                scr = tup.tile([128, 2048], f16, tag="tu")
                scr3 = scr.rearrange("p (r q) -> p r q", q=64)
                for i in range(3):
                    for j in range(3):
                        t = 3 * i + j
                        if j == 0:
                            in0 = qe3[:, i:i + 63:2, 0:64]
                        elif j == 2:
                            in0 = qe3[:, i:i + 63:2, 1:65]
                        else:
                            in0 = qo3[:, i:i + 63:2, 0:64]
                        nc.vector._custom_dve(
                            TENSOR_TENSOR_REDUCE,
                            out=scr3[:],
                            in0=in0,
                            in1=k3[:, 0:32, 0:64],
                            s0=sm[:, 68 + 9 * h + t:69 + 9 * h + t],
                            s1=SCALE,
                            accum_out=sm[:, 9 * h + t:9 * h + t + 1])
                nc.sync.dma_start(S_b[h][:], sm[:, 9 * h:9 * h + 9])
                if stage >= 3:
                    nc.gpsimd.collective_compute(
                        "AllReduce", ADD,
                        replica_groups=[[0, 1], [2, 3], [4, 5], [6, 7]],
                        ins=[S_b[h].opt()], outs=[S_r[h].opt()])
                    nc.sync.dma_start(sm[:, 18 + 9 * h:27 + 9 * h], S_r[h][:])

            if stage == 2:
                for h in range(2):
                    nc.sync.dma_start(out_d[128 * h:128 * h + 128, 0:9],
                                      sm[:, 9 * h:9 * h + 9])
            if stage == 3:
                for h in range(2):
                    nc.sync.dma_start(out_d[128 * h:128 * h + 128, 0:9],
                                      sm[:, 18 + 9 * h:27 + 9 * h])

            # ---- softmax + fold + final multiply per half ----
            for h in range(2 if stage >= 4 else 0):
                Ss = sm[:, 18 + 9 * h:27 + 9 * h]
                Av = sm[:, 36 + 9 * h:45 + 9 * h]
                Ev = sm[:, 54:63]
                nc.vector.tensor_reduce(sm[:, 63:64], Ss, axis=AX, op=MAX)
                nc.scalar.mul(sm[:, 64:65], sm[:, 63:64], -1.0)
                nc.scalar.activation(Ev, Ss, Exp, bias=sm[:, 64:65], scale=1.0)
                nc.vector.tensor_reduce(sm[:, 65:66], Ev, axis=AX, op=ADD)
                nc.vector.reciprocal(sm[:, 66:67], sm[:, 65:66])
                nc.vector.tensor_scalar(out=Av, in0=Ev,
                                        scalar1=sm[:, 66:67], scalar2=None,
                                        op0=MULT)

                def a(t):
                    return Av[:, t:t + 1]

                if stage < 5:
                    nc.sync.dma_start(out_d[128 * h:128 * h + 128, 16:25],
                                      sm[:, 36 + 9 * h:45 + 9 * h])
                    continue

                vc3 = vc_t[h].rearrange("p (r q) -> p r q", q=FEAT_C)
                src3 = src_t[h].rearrange("p (r q) -> p r q", q=SRC_C)
                # whole-half views: out rows x in [0,64), v rows m in [0,33)
                v00 = vc3[:, 0:32, 0:64]
                v01 = vc3[:, 0:32, 1:65]
                v10 = vc3[:, 1:33, 0:64]
                v11 = vc3[:, 1:33, 1:65]
                s11 = src3[:, 1:64:2, 1:128:2]
                s12 = src3[:, 1:64:2, 2:129:2]
                s21 = src3[:, 2:65:2, 1:128:2]
                s22 = src3[:, 2:65:2, 2:129:2]

                O = outp.tile([128, OUTN], f32, tag="O")
                O3 = O.rearrange("p (x y) -> p x y", y=128)
                Oee = O3[:, 0:63:2, 0:127:2]
                Oeo = O3[:, 0:63:2, 1:128:2]
                Ooe = O3[:, 1:64:2, 0:127:2]
                Ooo = O3[:, 1:64:2, 1:128:2]

                def v2(tl):
                    return tl.rearrange("p (r q) -> p r q", q=64)

                # ee: (v00 * A4) * src
                nc.vector.scalar_tensor_tensor(
                    out=Oee, in0=v00, scalar=a(4), in1=s11,
                    op0=MULT, op1=MULT)
                # eo: (A3*v01 + A5*v00) * src
                T1 = tup.tile([128, 2048], f16, tag="tu")
                nc.scalar.mul(v2(T1), v00, a(5))
                U1 = tup.tile([128, 2048], f16, tag="tu")
                nc.vector.scalar_tensor_tensor(
                    out=v2(U1), in0=v01, scalar=a(3), in1=v2(T1),
                    op0=MULT, op1=ADD)
                nc.gpsimd.tensor_tensor(out=Oeo, in0=v2(U1), in1=s12, op=MULT)
                # oe: (A1*v10 + A7*v00) * src
                T2 = tup.tile([128, 2048], f16, tag="tu")
                nc.scalar.mul(v2(T2), v10, a(1))
                U2 = tup.tile([128, 2048], f16, tag="tu")
                nc.vector.scalar_tensor_tensor(
                    out=v2(U2), in0=v00, scalar=a(7), in1=v2(T2),
                    op0=MULT, op1=ADD)
                nc.gpsimd.tensor_tensor(out=Ooe, in0=v2(U2), in1=s21, op=MULT)
                # oo: (A0*v11 + A2*v10 + A6*v01 + A8*v00) * src
                T3 = tup.tile([128, 2048], f16, tag="tu")
                nc.scalar.mul(v2(T3), v11, a(0))
                T4 = tup.tile([128, 2048], f16, tag="tu")
                nc.scalar.mul(v2(T4), v01, a(6))
                U3 = tup.tile([128, 2048], f16, tag="tu")
                nc.vector.scalar_tensor_tensor(
                    out=v2(U3), in0=v10, scalar=a(2), in1=v2(T3),
                    op0=MULT, op1=ADD)
                U4 = tup.tile([128, 2048], f16, tag="tu")
                nc.vector.scalar_tensor_tensor(
                    out=v2(U4), in0=v00, scalar=a(8), in1=v2(T4),
                    op0=MULT, op1=ADD)
                U5 = tup.tile([128, 2048], f16, tag="tu")
                nc.vector.tensor_tensor(out=v2(U5), in0=v2(U3), in1=v2(U4),
                                        op=ADD)
                nc.gpsimd.tensor_tensor(out=Ooo, in0=v2(U5), in1=s22, op=MULT)

                nc.sync.dma_start(out_d[128 * h:128 * h + 128, :], O[:])

    nc.compile()
    return nc


def _get_program(add_bv: bool, stage: int = 99):
    key = (add_bv, stage)
    if key not in _prog_cache:
        _prog_cache[key] = _build(add_bv, stage)
    return _prog_cache[key]


def kernel(feat, src, Wq, bq, Wv, bv):
    from concourse.bass_utils import run_bass_kernel_spmd

    feat = np.ascontiguousarray(np.asarray(feat, dtype=np.float32))
    src = np.ascontiguousarray(np.asarray(src, dtype=np.float32))
    Wq = np.asarray(Wq, dtype=np.float32)
    bq = np.asarray(bq, dtype=np.float32)
    Wv = np.asarray(Wv, dtype=np.float32)
    bv = np.asarray(bv, dtype=np.float32)
    B, C, H, W = src.shape

    src_pad = np.pad(src, ((0, 0), (0, 0), (1, 1), (1, 1)))
    feat_pad = np.pad(feat, ((0, 0), (0, 0), (0, 1), (0, 2)))
    wpack = np.ascontiguousarray(
        np.concatenate([Wq.T, Wv.T], axis=1).astype(np.float32))

    add_bv = bool(np.any(bv))
    nc = _get_program(add_bv, STAGE[0])

    # bq correction seeds for the q.k reduction: S += bq * sum(valid k) * scale
    sinits = {}
    if np.any(bq):
        for b in range(B):
            for s in range(2):
                k = feat[b, :, 32 * s:32 * s + 32, :]
                corr = np.zeros((C, 9), np.float32)
                for i in range(3):
                    for j in range(3):
                        valid = np.ones((32, 64), bool)
                        if i == 0 and s == 0:
                            valid[0, :] = False
                        if j == 0:
                            valid[:, 0] = False
                        corr[:, 3 * i + j] = bq * (k * valid).sum((1, 2)) * SCALE
                sinits[(b, s)] = corr
    zero_sinit = np.zeros((C, 9), np.float32)

    in_maps = []
    for core in range(N_CORES):
        b, s = core // 2, core % 2
        src_slab = np.ascontiguousarray(
            src_pad[b, :, 64 * s:64 * s + SRC_R, :SRC_C].reshape(C, SRCN))
        feat_slab = np.ascontiguousarray(
            feat_pad[b, :, 32 * s:32 * s + FEAT_R, :FEAT_C].reshape(C, FEATN))
        in_maps.append({
            "src": src_slab,
            "feat": feat_slab,
            "wpack": wpack,
            "s_init": sinits.get((b, s), zero_sinit),
            "bv": bv.reshape(C, 1),
        })

    res = run_bass_kernel_spmd(nc, in_maps, list(range(N_CORES)),
                               trace=TRACE, **TRACE_KW)
    LAST_RESULT[0] = res

    out = np.empty((B, C, H, W), np.float32)
    for core in range(N_CORES):
        b, s = core // 2, core % 2
        out[b, :, 64 * s:64 * s + 64, :] = \
            res.results[core]["out"].reshape(C, 64, 128)
    return out



# revision 3
# speedup vs baseline: 1.0666x; 1.0666x over previous
"""Trainium2 Bass kernel for nn_CRAU (per-channel sparse attention), v3.

Computation (per batch b, channel c):
  qc  = Wq @ src (1x1 conv; bias folded into the S-reduction seed)
  S[c,t] = sum_d unfold(qc)[c,t,d] * feat[c,d] * (1/64)      t in 3x3 window
  A   = softmax_t(S)   (no max subtraction: |S| <~ 4, exp safe in f16)
  vc  = Wv @ feat + bv (1x1 conv)
  out = fold(A outer vc) * src

Sharding: 8 cores = 4 batches x 2 spatial halves (rows), pairwise AllReduce
of the partial S ([128,9] f32 per channel-half).

Layout: src ships f16 in PARITY-PLANE order (padded slab split into
ee/eo/oe/oo row/col-parity planes) so the q-conv emits qc directly in parity
order, every TTR window is a contiguous [32,64] block, and the fold's src
reads are contiguous.  The fold runs on the PE as diag(A_t) matmuls
accumulating each output-parity quarter in PSUM (diag built via gpsimd
local_scatter from normalized softmax in f16); Scalar drains F to SBUF f16
and the final F*src multiply runs on GpSimd for h0 (hidden under TTR h1 /
AllReduce h1) and on DVE for h1 (short tail).  Output is written f16 as
parity quarters and reassembled host-side.
"""

import numpy as np

N_CORES = 8
SRC_R, SRC_C = 65, 129          # per-core src slab (padded rows/cols)
FEAT_R, FEAT_C = 33, 66         # per-core feat slab (padded, even width)
SRCN = SRC_R * SRC_C            # 8385
FEATN = FEAT_R * FEAT_C         # 2178
OUTN = 64 * 128                 # per-core output elements per channel (8192)
SCALE = 1.0 / 64.0

# parity planes of the padded src slab, in [ee, eo, oe, oo] order
PL_SHAPES = [(33, 65), (33, 64), (32, 65), (32, 64)]
PL_SIZES = [r * c for r, c in PL_SHAPES]          # 2145, 2112, 2080, 2048
PL_OFF = [0, 2145, 4257, 6337]

# offset t=3i+j -> (plane index, row offset, col offset) in qc parity planes
T_PLANE = {
    0: (0, 0, 0), 1: (1, 0, 0), 2: (0, 0, 1),
    3: (2, 0, 0), 4: (3, 0, 0), 5: (2, 0, 1),
    6: (0, 1, 0), 7: (1, 1, 0), 8: (0, 1, 1),
}
# TTR groups per plane (plane -> list of t)
PLANE_TS = {pl: [t for t in range(9) if T_PLANE[t][0] == pl]
            for pl in range(4)}
# S columns stored plane-major so the cross-core exchange can ship the
# ee+eo planes' offsets (cols 0:6) while the oe/oo TTRs still run
PERM = [0, 2, 6, 8, 1, 7, 3, 5, 4]
POS = {t: i for i, t in enumerate(PERM)}

_prog_cache = {}
TRACE = False
TRACE_KW = {}
LAST_RESULT = [None]
STAGE = [99]


def _build(add_bv: bool, stage: int = 99):
    import concourse.mybir as mybir
    import concourse.tile as tile
    from concourse import bacc
    from concourse.dve_ops import TENSOR_TENSOR_REDUCE

    f32 = mybir.dt.float32
    f16 = mybir.dt.float16
    i16 = mybir.dt.int16
    ADD = mybir.AluOpType.add
    MULT = mybir.AluOpType.mult
    AX = mybir.AxisListType.X
    Exp = mybir.ActivationFunctionType.Exp
    Copy = mybir.ActivationFunctionType.Copy

    nc = bacc.Bacc("TRN2", target_bir_lowering=False, debug=False,
                   num_devices=N_CORES)

    src_d = nc.dram_tensor("src", [256, SRCN], f16, kind="ExternalInput").ap()
    feat_d = nc.dram_tensor("feat", [256, FEATN], f16, kind="ExternalInput").ap()
    wpack_d = nc.dram_tensor("wpack", [256, 512], f16, kind="ExternalInput").ap()
    sinit_d = nc.dram_tensor("s_init", [256, 9], f32, kind="ExternalInput").ap()
    bv_d = nc.dram_tensor("bv", [256, 1], f32, kind="ExternalInput").ap()
    idx_d = nc.dram_tensor("idx", [256, 2], i16, kind="ExternalInput").ap()
    out_d = nc.dram_tensor("out", [256, OUTN], f16, kind="ExternalOutput").ap()

    with tile.TileContext(nc) as tc:
        with (
            tc.tile_pool(name="srcp", bufs=2) as srcp,
            tc.tile_pool(name="featp", bufs=2) as featp,
            tc.tile_pool(name="vcp", bufs=2) as vcp,
            tc.tile_pool(name="qcp", bufs=2) as qcp,
            tc.tile_pool(name="constp", bufs=2) as constp,
            tc.tile_pool(name="smp", bufs=1) as smp,
            tc.tile_pool(name="scrp", bufs=2) as scrp,
            tc.tile_pool(name="fsbp", bufs=4) as fsbp,
            tc.tile_pool(name="diagp", bufs=2) as diagp,
            tc.tile_pool(name="outp", bufs=2) as outp,
            tc.tile_pool(name="psc", bufs=2, space="PSUM") as psc,
            tc.tile_pool(name="psf", bufs=2, space="PSUM") as psf,
            tc.tile_pool(name="dramp", bufs=2, space="DRAM") as dramp,
        ):
            # smalls layout (f32 cols):
            # [0:9] S(h0) [9:18] S(h1) [18:27] Ssum(h0) [27:36] Ssum(h1)
            # [42:43] esum(h0) [43:44] esum(h1) [44:45] recip(h0) [45:46] r(h1)
            # [48:57] sinit(h0) [57:66] sinit(h1) [66:68] bv(h0,h1)
            sm = smp.tile([128, 72], f32, tag="smalls")
            # E/A f16: [0:10] E(h0) [10:20] E(h1) [20:30] A(h0) [30:40] A(h1)
            am = smp.tile([128, 40], f16, tag="am")
            idxt = smp.tile([128, 2], i16, tag="idx")

            # ---- loads (src h0-ee plane first: it gates the critical
            # conv->TTR pipeline start) ----
            # trigger issuance is ~0.7us of engine time per dma_start, so
            # spread the load triggers across sync/scalar/gpsimd queues
            w_t = []
            for h in range(2):
                wt = constp.tile([128, 512], f16, tag="w")
                nc.gpsimd.dma_start(wt[:], wpack_d[128 * h:128 * h + 128, :])
                w_t.append(wt)
            src_t = []
            for h in range(2):
                st = srcp.tile([128, SRCN], f16, tag="src")
                src_t.append(st)

            def load_src_plane(h, pl, eng):
                o0, sz = PL_OFF[pl], PL_SIZES[pl]
                eng.dma_start(
                    src_t[h][:, o0:o0 + sz],
                    src_d[128 * h:128 * h + 128, o0:o0 + sz])

            for pl in range(4):
                load_src_plane(0, pl, nc.sync)
            feat_t = []
            for h in range(2):
                ft = featp.tile([128, FEATN], f16, tag="feat")
                nc.scalar.dma_start(ft[:], feat_d[128 * h:128 * h + 128, :])
                feat_t.append(ft)
            for pl in range(4):
                load_src_plane(1, pl, nc.gpsimd)

            # warm up the CC stream: the first collective of a NEFF pays an
            # ~11us start delay on top of the launch barrier; burn it here on
            # dummy data so AR0 starts promptly once S(h0) is ready
            if stage >= 3:
                wu_b = dramp.tile([128, 1], f32, tag="wub", name="wub")
                wu_r = dramp.tile([128, 1], f32, tag="wur", name="wur")
                nc.vector.memset(sm[:, 71:72], 0.0)
                nc.sync.dma_start(wu_b[:], sm[:, 71:72])
                nc.gpsimd.collective_compute(
                    "AllReduce", ADD,
                    replica_groups=[[0, 1], [2, 3], [4, 5], [6, 7]],
                    ins=[wu_b.opt()], outs=[wu_r.opt()])


            nc.scalar.dma_start(idxt[:], idx_d[0:128, :])
            for h in range(2):
                nc.scalar.dma_start(sm[:, 48 + 9 * h:57 + 9 * h],
                                    sinit_d[128 * h:128 * h + 128, :])
                if add_bv:
                    nc.scalar.dma_start(sm[:, 66 + h:67 + h],
                                        bv_d[128 * h:128 * h + 128, :])

            # ---- v-conv (PE) + Scalar bias-drain; emitted via helper so it
            # can run AFTER the q-convs (vc is only needed by the fold) ----
            vc_t = []

            def vconv(h):
                vt = vcp.tile([128, FEATN], f16, tag="vc")
                for c0 in range(0, FEATN, 1024):
                    csz = min(1024, FEATN - c0)
                    pt = psc.tile([128, 1024], f32, tag="mmc")
                    for kt in range(2):
                        for s0 in range(0, csz, 512):
                            ssz = min(512, csz - s0)
                            nc.tensor.matmul(
                                pt[:, s0:s0 + ssz],
                                lhsT=w_t[kt][:, 256 + 128 * h:256 + 128 * h + 128],
                                rhs=feat_t[kt][:, c0 + s0:c0 + s0 + ssz],
                                start=(kt == 0), stop=(kt == 1))
                    if add_bv:
                        nc.scalar.activation(
                            vt[:, c0:c0 + csz], pt[:, 0:csz], Copy,
                            bias=sm[:, 66 + h:67 + h], scale=1.0)
                    else:
                        nc.scalar.copy(vt[:, c0:c0 + csz], pt[:, 0:csz])
                if add_bv:
                    v3 = vt.rearrange("p (r q) -> p r q", q=FEAT_C)
                    nc.vector.memset(v3[:, FEAT_R - 1, :], 0.0)
                    nc.vector.memset(v3[:, :, 64:66], 0.0)
                vc_t.append(vt)

            S_b = {}
            S_r = {}
            for h in range(2):
                sb = dramp.tile([128, 9], f32, tag=f"sb{h}", name=f"sb{h}")
                sr = dramp.tile([128, 9], f32, tag=f"sr{h}", name=f"sr{h}")
                S_b[(h, "A")] = sb
                S_r[(h, "A")] = sr

            # ---- helpers ----
            def qconv_plane(h, qt, pl):
                """conv one parity plane of half h into SBUF qc tile."""
                o0, sz = PL_OFF[pl], PL_SIZES[pl]
                for c0 in range(0, sz, 1024):
                    csz = min(1024, sz - c0)
                    pt = psc.tile([128, 1024], f32, tag="mmc")
                    for kt in range(2):
                        for s0 in range(0, csz, 512):
                            ssz = min(512, csz - s0)
                            nc.tensor.matmul(
                                pt[:, s0:s0 + ssz],
                                lhsT=w_t[kt][:, 128 * h:128 * h + 128],
                                rhs=src_t[kt][:, o0 + c0 + s0:o0 + c0 + s0 + ssz],
                                start=(kt == 0), stop=(kt == 1))
                    nc.scalar.copy(qt[:, o0 + c0:o0 + c0 + csz], pt[:, 0:csz])

            def ttr_plane(h, qpl3, k3, scr3, pl):
                for t in PLANE_TS[pl]:
                    _, dr, dc = T_PLANE[t]
                    p = POS[t]
                    nc.vector._custom_dve(
                        TENSOR_TENSOR_REDUCE,
                        out=scr3[:],
                        in0=qpl3[pl][:, dr:dr + 32, dc:dc + 64],
                        in1=k3[:, 0:32, 0:64],
                        s0=sm[:, 48 + 9 * h + p:49 + 9 * h + p],
                        s1=SCALE,
                        accum_out=sm[:, 9 * h + p:9 * h + p + 1])

            def softmax_scalar(h):
                # exp(Ssum) -> E (f16), no max subtraction
                nc.scalar.activation(am[:, 10 * h:10 * h + 9],
                                     sm[:, 18 + 9 * h:27 + 9 * h], Exp)

            def softmax_dve(h):
                Ev = am[:, 10 * h:10 * h + 9]
                nc.vector.tensor_reduce(sm[:, 42 + h:43 + h], Ev, axis=AX,
                                        op=ADD)
                nc.vector.reciprocal(sm[:, 44 + h:45 + h], sm[:, 42 + h:43 + h])
                nc.vector.tensor_scalar(
                    out=am[:, 20 + 10 * h:29 + 10 * h], in0=Ev,
                    scalar1=sm[:, 44 + h:45 + h], scalar2=None, op0=MULT)

            def scatters(h):
                dgs = []
                for t in range(9):
                    dg = diagp.tile([128, 128], f16, tag=f"diag{t}")
                    p = POS[t]
                    nc.gpsimd.local_scatter(
                        dg[:], am[:, 20 + 10 * h + p:22 + 10 * h + p], idxt[:],
                        channels=128, num_elems=128, num_idxs=2)
                    dgs.append(dg)
                return dgs

            def fold_half(h, diags, O):
                vc3 = vc_t[h].rearrange("p (r q) -> p r q", q=FEAT_C)
                src3 = [src_t[h][:, PL_OFF[p]:PL_OFF[p] + PL_SIZES[p]].rearrange(
                            "p (r q) -> p r q", q=PL_SHAPES[p][1])
                        for p in range(4)]
                O3 = O.rearrange("p (x y) -> p x y", y=64)   # [128, 128, 64]
                quarters = [
                    ([(4, 0, 0)], src3[3][:, 0:32, 0:64]),
                    ([(3, 0, 1), (5, 0, 0)], src3[2][:, 0:32, 1:65]),
                    ([(1, 1, 0), (7, 0, 0)], src3[1][:, 1:33, 0:64]),
                    ([(0, 1, 1), (2, 1, 0), (6, 0, 1), (8, 0, 0)],
                     src3[0][:, 1:33, 1:65]),
                ]
                subq = 0
                for qi, (terms, sv) in enumerate(quarters):
                    for u0 in range(0, 32, 16):
                        pf = psf.tile([128, 1024], f32, tag="mmf")
                        for ti, (t, dr, dc) in enumerate(terms):
                            rhs = vc3[:, u0 + dr:u0 + dr + 16, dc:dc + 64]
                            for s0 in range(0, 1024, 512):
                                nc.tensor.matmul(
                                    pf[:, s0:s0 + 512],
                                    lhsT=diags[t][:],
                                    rhs=rhs[:, s0 // 64:(s0 + 512) // 64, :],
                                    start=(ti == 0), stop=(ti == len(terms) - 1))
                        fsb = fsbp.tile([128, 1024], f16, tag="fsb")
                        nc.scalar.copy(fsb[:], pf[:])
                        fsb3 = fsb.rearrange("p (r q) -> p r q", q=64)
                        # h0: early finals on gpsimd, tail ones on DVE (free
                        # once TTR h1 drains). h1: all on DVE (short tail).
                        eng = nc.gpsimd if (h == 0 and subq < 5) else nc.vector
                        eng.tensor_tensor(
                            out=O3[:, 32 * qi + u0:32 * qi + u0 + 16, :],
                            in0=fsb3[:],
                            in1=sv[:, u0:u0 + 16, :],
                            op=MULT)
                        subq += 1
                    dma_eng = nc.gpsimd if h == 0 else nc.sync
                    dma_eng.dma_start(
                        out_d[128 * h:128 * h + 128,
                              2048 * qi:2048 * qi + 2048],
                        O[:, 2048 * qi:2048 * qi + 2048])

            # ---- q-conv h0 + TTR h0, then AR0 ----
            qc_t = []
            scr_t = []
            kq = []
            for h in range(2 if stage >= 2 else 0):
                qt = qcp.tile([128, SRCN], f16, tag="qc")
                qpl3 = [qt[:, PL_OFF[p]:PL_OFF[p] + PL_SIZES[p]].rearrange(
                            "p (r q) -> p r q", q=PL_SHAPES[p][1])
                        for p in range(4)]
                k3 = feat_t[h].rearrange("p (r q) -> p r q", q=FEAT_C)
                scr = scrp.tile([128, 2048], f16, tag="scr")
                scr3 = scr.rearrange("p (r q) -> p r q", q=64)
                qc_t.append((qt, qpl3))
                scr_t.append(scr3)
                kq.append(k3)

            def ar(h):
                c0 = 9 * h
                nc.sync.dma_start(S_b[(h, "A")][:], sm[:, c0:c0 + 9])
                if stage >= 3:
                    nc.gpsimd.collective_compute(
                        "AllReduce", ADD,
                        replica_groups=[[0, 1], [2, 3], [4, 5], [6, 7]],
                        ins=[S_b[(h, "A")].opt()], outs=[S_r[(h, "A")].opt()])
                    nc.sync.dma_start(sm[:, 18 + c0:27 + c0],
                                      S_r[(h, "A")][:])

            diags0_box = []
            if stage >= 2:
                qt, qpl3 = qc_t[0]
                for pl in range(4):
                    qconv_plane(0, qt, pl)
                    ttr_plane(0, qpl3, kq[0], scr_t[0], pl)
                ar(0)

                # ---- q-conv h1 + TTR h1; softmax h0 interleaved ----
                qt1, qpl31 = qc_t[1]
                for pl in range(4):
                    qconv_plane(1, qt1, pl)
                    ttr_plane(1, qpl31, kq[1], scr_t[1], pl)
                    if pl == 2 and stage >= 4:
                        # h0 softmax + scatters, mid-TTR-h1 so fold h0
                        # overlaps the TTR h1 / exchange window
                        softmax_scalar(0)
                        softmax_dve(0)
                        if stage >= 5:
                            diags0_box.append(scatters(0))
                ar(1)

            # v-conv emitted after the q-convs: vc is only needed by the
            # fold, and this keeps the PE on the critical TTR feed early
            for h in range(2 if stage >= 1 else 0):
                vconv(h)

            if stage == 2:
                for h in range(2):
                    nc.gpsimd.dma_start(out_d[128 * h:128 * h + 128, 0:9],
                                        sm[:, 9 * h:9 * h + 9])
            if stage == 3:
                for h in range(2):
                    nc.gpsimd.dma_start(out_d[128 * h:128 * h + 128, 0:9],
                                        sm[:, 18 + 9 * h:27 + 9 * h])

            if stage >= 5:
                diags0 = diags0_box[0]
                # softmax h1 + its scatters BEFORE fold h0's emission: they
                # only wait on AR1, and this keeps them ahead of fold h0's
                # slow gpsimd finals in the queues so fold h1 starts promptly
                softmax_scalar(1)
                softmax_dve(1)
                diags1 = scatters(1)
                O0 = outp.tile([128, OUTN], f16, tag="O")
                fold_half(0, diags0, O0)
                O1 = outp.tile([128, OUTN], f16, tag="O")
                fold_half(1, diags1, O1)
            elif stage == 4:
                softmax_scalar(1)
                softmax_dve(1)
                for h in range(2):
                    nc.gpsimd.dma_start(out_d[128 * h:128 * h + 128, 16:25],
                                        am[:, 10 * h:10 * h + 9])

    nc.compile()
    return nc


def _get_program(add_bv: bool, stage: int = 99):
    key = (add_bv, stage)
    if key not in _prog_cache:
        _prog_cache[key] = _build(add_bv, stage)
    return _prog_cache[key]


def _parity_pack(slab):
    """[C, 65, 129] -> [C, 8385] in [ee|eo|oe|oo] plane order."""
    C = slab.shape[0]
    return np.concatenate([
        slab[:, 0::2, 0::2].reshape(C, -1),
        slab[:, 0::2, 1::2].reshape(C, -1),
        slab[:, 1::2, 0::2].reshape(C, -1),
        slab[:, 1::2, 1::2].reshape(C, -1),
    ], axis=1)


def kernel(feat, src, Wq, bq, Wv, bv):
    from concourse.bass_utils import run_bass_kernel_spmd

    feat = np.ascontiguousarray(np.asarray(feat, dtype=np.float32))
    src = np.ascontiguousarray(np.asarray(src, dtype=np.float32))
    Wq = np.asarray(Wq, dtype=np.float32)
    bq = np.asarray(bq, dtype=np.float32)
    Wv = np.asarray(Wv, dtype=np.float32)
    bv = np.asarray(bv, dtype=np.float32)
    B, C, H, W = src.shape

    src_pad = np.pad(src, ((0, 0), (0, 0), (1, 1), (1, 1)))
    feat_pad = np.pad(feat, ((0, 0), (0, 0), (0, 1), (0, 2)))
    wpack = np.ascontiguousarray(
        np.concatenate([Wq.T, Wv.T], axis=1).astype(np.float16))

    add_bv = bool(np.any(bv))
    nc = _get_program(add_bv, STAGE[0])

    # bq correction seeds for the q.k reduction: S += bq * sum(valid k) * scale
    sinits = {}
    if np.any(bq):
        for b in range(B):
            for s in range(2):
                k = feat[b, :, 32 * s:32 * s + 32, :]
                corr = np.zeros((C, 9), np.float32)
                for i in range(3):
                    for j in range(3):
                        valid = np.ones((32, 64), bool)
                        if i == 0 and s == 0:
                            valid[0, :] = False
                        if j == 0:
                            valid[:, 0] = False
                        corr[:, 3 * i + j] = bq * (k * valid).sum((1, 2)) * SCALE
                sinits[(b, s)] = np.ascontiguousarray(corr[:, PERM])
    zero_sinit = np.zeros((C, 9), np.float32)

    idx = np.zeros((C, 2), np.int16)
    idx[:, 0] = np.arange(C) % 128
    idx[:, 1] = -1

    in_maps = []
    for core in range(N_CORES):
        b, s = core // 2, core % 2
        src_slab = _parity_pack(
            src_pad[b, :, 64 * s:64 * s + SRC_R, :SRC_C]).astype(np.float16)
        feat_slab = np.ascontiguousarray(
            feat_pad[b, :, 32 * s:32 * s + FEAT_R, :FEAT_C].reshape(
                C, FEATN)).astype(np.float16)
        in_maps.append({
            "src": np.ascontiguousarray(src_slab),
            "feat": feat_slab,
            "wpack": wpack,
            "s_init": sinits.get((b, s), zero_sinit),
            "bv": bv.reshape(C, 1),
            "idx": idx,
        })

    res = run_bass_kernel_spmd(nc, in_maps, list(range(N_CORES)),
                               trace=TRACE, **TRACE_KW)
    LAST_RESULT[0] = res

    out = np.empty((B, C, H, W), np.float32)
    for core in range(N_CORES):
        b, s = core // 2, core % 2
        o = res.results[core]["out"].reshape(C, 4, 32, 64).astype(np.float32)
        r0 = 64 * s
        out[b, :, r0 + 0:r0 + 64:2, 0::2] = o[:, 0]
        out[b, :, r0 + 0:r0 + 64:2, 1::2] = o[:, 1]
        out[b, :, r0 + 1:r0 + 64:2, 0::2] = o[:, 2]
        out[b, :, r0 + 1:r0 + 64:2, 1::2] = o[:, 3]
    return out
